# revision 16
# baseline (speedup 1.0000x reference)
"""Trainium2 Bass kernel for bidirectional gated linear recurrence block.

Reference computation (per spatial position, channel-mixing MLPs):
  Z = tanh(W_z2 @ tanh(W_z1 @ x + b_z1) + b_z2)
  F = sigmoid(W_f2 @ tanh(W_f1 @ x + b_f1) + b_f2)
  channels 0:32  : h_t = F*h_{t-1} + (1-F)*Z forward over T
  channels 32:64 : same recurrence backward over T

Sharding: H (=64) split across 8 cores, 8 rows each. Everything else is
per-position so no collectives are needed.

Per-core dataflow (all fp32):
  - tiles of (b, h-pair): x_tile [64c, t=32, s=128] where s=(h2,w), DMA with
    512B-contiguous DRAM runs.
  - L1 matmul per 512-position block: lhsT = [Wz1;Wf1]^T [64,128],
    rhs = x strided (s-outer, t-inner) -> PSUM [128, 512] (h1z | h1f).
  - tanh via ScalarE (bias folded) -> h1 SBUF.
  - L2: col-tiled matmuls pack TWO blocks (P,Q) onto 128 PSUM partitions:
    psZ rows = [zP_fwd, zQ_fwd, zP_bwd, zQ_bwd] (32 chans each), psF same
    for the gate. Gate branch pre-scaled by 0.5 so one tanh pass gives
    t = tanh(m/2); then f = 0.5+0.5t, 1-f = 0.5-0.5t.
  - DVE: a = 0.5u+0.5 (zeroed at each segment's first step), g=(0.5-0.5u)*z,
    y = tensor_tensor_scan(a, g) along (s,t) free dim; bwd rows use
    negative-stride APs so the same forward scan implements reversed time.
  - DMA out per 32-row slice.
"""

import numpy as np

B, C, T, H, W = 2, 64, 32, 64, 64
NCORES = 8
HL = H // NCORES          # 8 h-rows per core
HP = HL // 2              # 4 h-pair tiles per (b)
S = 2 * W                 # 128 positions per h-pair (h2, w)
NBLK = S // 16            # 8 blocks of 512 (=16 s * 32 t) per tile
CH = C // 2               # 32 = fwd (or bwd) channel count

_built = {}


def _build():
    import concourse.bass as bass
    import concourse.mybir as mybir
    import concourse.tile as tile
    from concourse import bacc

    fp32 = mybir.dt.float32
    f32r = mybir.dt.float32r
    nc = bacc.Bacc(None, target_bir_lowering=False)

    x = nc.dram_tensor("x", [B, C, T, HL, W], mybir.dt.float32r, kind="ExternalInput")
    w1catT = nc.dram_tensor("w1catT", [C, 2 * C], mybir.dt.float32r, kind="ExternalInput")
    w2blk = nc.dram_tensor("w2blk", [2 * C, 2 * C], mybir.dt.bfloat16, kind="ExternalInput")
    b1cat = nc.dram_tensor("b1cat", [2 * C, 1], fp32, kind="ExternalInput")
    bz2p = nc.dram_tensor("bz2p", [2 * C, 1], fp32, kind="ExternalInput")
    bf2p = nc.dram_tensor("bf2p", [2 * C, 1], fp32, kind="ExternalInput")
    y = nc.dram_tensor("y", [B, C, T, HL, W], fp32, kind="ExternalOutput")

    MUL = mybir.AluOpType.mult
    ADD = mybir.AluOpType.add
    TANH = mybir.ActivationFunctionType.Tanh

    def rev2d(ap2d):
        # Reverse the free dim of a 2D [P, F] contiguous AP (step 1 -> -1).
        (pstep, pcnt), (fstep, fcnt) = [list(d) for d in ap2d.ap]
        assert fstep == 1, ap2d.ap
        return bass.AP(
            tensor=ap2d.tensor,
            offset=ap2d.offset + (fcnt - 1),
            ap=[[pstep, pcnt], [-1, fcnt]],
        )

    with tile.TileContext(nc) as tc:
        with (
            tc.tile_pool(name="consts", bufs=1) as consts,
            tc.tile_pool(name="xin", bufs=2) as xin,
            tc.tile_pool(name="h1p", bufs=3) as h1p,
            tc.tile_pool(name="ew", bufs=3) as ew,
            tc.tile_pool(name="yout", bufs=3) as yout,
            tc.tile_pool(name="psH", bufs=2, space="PSUM") as psH,
            tc.tile_pool(name="psZ", bufs=2, space="PSUM") as psZ,
            tc.tile_pool(name="psF", bufs=2, space="PSUM") as psF,
        ):
            w1_sb = consts.tile([C, 2 * C], f32r)
            nc.sync.dma_start(out=w1_sb, in_=w1catT[:, :])
            w2_sb = consts.tile([2 * C, 2 * C], mybir.dt.bfloat16)
            nc.sync.dma_start(out=w2_sb, in_=w2blk[:, :])
            b1_sb = consts.tile([2 * C, 1], fp32)
            nc.sync.dma_start(out=b1_sb, in_=b1cat[:, :])
            bz2_sb = consts.tile([2 * C, 1], fp32)
            nc.sync.dma_start(out=bz2_sb, in_=bz2p[:, :])
            bf2_sb = consts.tile([2 * C, 1], fp32)
            nc.sync.dma_start(out=bf2_sb, in_=bf2p[:, :])

            for b in range(B):
                for hp in range(HP):
                    x_t = xin.tile([C, T, S], f32r)
                    nc.sync.dma_start(
                        out=x_t,
                        in_=x[b, :, :, 2 * hp : 2 * hp + 2, :].rearrange(
                            "c t h w -> c t (h w)"
                        ),
                    )
                    # full-tile staging buffer in (t, w) order: rows
                    # [c0:32@h0, c0:32@h1, c32:64@h0, c32:64@h1]
                    y_ts = yout.tile([2 * C, T, W], fp32, tag="yts")
                    # 4 block-pairs: P = s[16j,16j+16) (h-row 0),
                    # Q = s[64+16j, 64+16j+16) (h-row 1)
                    for j in range(NBLK // 2):
                        pH = psH.tile([2 * C, 2, 512], fp32)
                        for q in range(2):
                            s0 = 64 * q + 16 * j
                            rhs = x_t[:, :, s0 : s0 + 16].rearrange(
                                "c t s -> c s t"
                            )
                            nc.tensor.matmul(
                                pH[:, q, :], w1_sb[:, :], rhs,
                                start=True, stop=True,
                            )
                        h1 = h1p.tile([2 * C, 2, 512], mybir.dt.bfloat16)
                        nc.scalar.activation(
                            h1.rearrange("p a n -> p (a n)"),
                            pH.rearrange("p a n -> p (a n)"),
                            TANH, bias=b1_sb[:, :],
                        )
                        pZ = psZ.tile([2 * C, 512], fp32)
                        pF = psF.tile([2 * C, 512], fp32)
                        # col-tiled L2 (f32r needs 64-aligned dst): pack the
                        # two blocks P,Q onto the 128 PSUM partitions:
                        # rows of psZ/psF = [P(ch 0:64), Q(ch 0:64)]
                        for q in range(2):
                            col = 64 * q
                            nc.tensor.matmul(
                                pZ[col : col + C, :],
                                w2_sb[0:C, 0:C], h1[0:C, q, :],
                                start=True, stop=True,
                                tile_position=(0, col),
                            )
                            nc.tensor.matmul(
                                pF[col : col + C, :],
                                w2_sb[C : 2 * C, C : 2 * C], h1[C : 2 * C, q, :],
                                start=True, stop=True,
                                tile_position=(64, col),
                            )
                        z_sb = ew.tile([2 * C, 16, 32], fp32, tag="z")
                        u_sb = ew.tile([2 * C, 16, 32], fp32, tag="u")
                        nc.scalar.activation(
                            z_sb.rearrange("p s t -> p (s t)"), pZ[:, :],
                            TANH, bias=bz2_sb[:, :],
                        )
                        nc.scalar.activation(
                            u_sb.rearrange("p s t -> p (s t)"), pF[:, :],
                            TANH, bias=bf2_sb[:, :],
                        )
                        u2 = u_sb.rearrange("p s t -> p (s t)")
                        z2 = z_sb.rearrange("p s t -> p (s t)")
                        a_sb = ew.tile([2 * C, 16, 32], fp32, tag="a")
                        g_sb = ew.tile([2 * C, 16, 32], fp32, tag="g")
                        a2 = a_sb.rearrange("p s t -> p (s t)")
                        g2 = g_sb.rearrange("p s t -> p (s t)")
                        # a = 0.5*u + 0.5 (= f) on GpSimd (DVE is the
                        # bottleneck engine; Pool is mostly idle)
                        nc.gpsimd.tensor_scalar(a2, u2, 0.5, 0.5, MUL, ADD)
                        # zero the coefficient at each segment's first
                        # step; fwd chans are rows [0:32] and [64:96],
                        # bwd chans rows [32:64] and [96:128]
                        nc.gpsimd.memset(a_sb[0:CH, :, 0], 0.0)
                        nc.gpsimd.memset(a_sb[CH:C, :, 31], 0.0)
                        nc.gpsimd.memset(a_sb[C : C + CH, :, 0], 0.0)
                        nc.gpsimd.memset(a_sb[C + CH :, :, 31], 0.0)
                        # g' = (u - 1) * z = -2*(1-f)*z; scan is linear in g
                        # so it yields -2*h, rescaled by -0.5 at relayout
                        nc.vector.scalar_tensor_tensor(
                            g2, u2, 1.0, z2, mybir.AluOpType.subtract, MUL
                        )
                        y_sb = yout.tile([2 * C, 16, 32], fp32, tag="ysb")
                        y2 = y_sb.rearrange("p s t -> p (s t)")
                        for r0 in (0, C):
                            nc.vector.tensor_tensor_scan(
                                y2[r0 : r0 + CH, :], a2[r0 : r0 + CH, :],
                                g2[r0 : r0 + CH, :], 0.0, MUL, ADD,
                            )
                            nc.vector.tensor_tensor_scan(
                                rev2d(y2[r0 + CH : r0 + C, :]),
                                rev2d(a2[r0 + CH : r0 + C, :]),
                                rev2d(g2[r0 + CH : r0 + C, :]),
                                0.0, MUL, ADD,
                            )
                        # relayout (s,t)->(t,s) + rescale by -0.5 on GpSimd
                        nc.gpsimd.tensor_scalar(
                            y_ts[:, :, 16 * j : 16 * j + 16],
                            y_sb.rearrange("p s t -> p t s"),
                            -0.5, None, MUL,
                        )
                    # store: 4 DMAs per tile, 256B-contiguous DRAM runs
                    # y_ts rows = [P ch0:32 | P ch32:64 | Q ch0:32 | Q ch32:64]
                    for q in range(2):
                        for half in range(2):
                            r0 = 64 * q + 32 * half
                            nc.sync.dma_start(
                                out=y[
                                    b,
                                    CH * half : CH * half + CH,
                                    :,
                                    2 * hp + q,
                                    :,
                                ],
                                in_=y_ts[r0 : r0 + 32, :, :],
                            )
    nc.compile()
    return nc


def _prep_weights(wz1, bz1, wz2, bz2, wf1, bf1, wf2, bf2):
    f32 = np.float32
    w1catT = np.ascontiguousarray(
        np.concatenate([wz1, wf1], axis=0).T, dtype=f32
    )  # [64, 128]
    import ml_dtypes
    w2blk = np.zeros((2 * C, 2 * C), dtype=f32)
    # rows 0:64 = h1z contraction, cols: [z_fwd(P-col 0:32 uses 0:32.. same
    # weights reused for Q via tile_position], layout: cols 0:32 z_fwd,
    # 32:64 z_bwd; rows 64:128 cols 64:96 f_fwd, 96:128 f_bwd (0.5-scaled)
    w2blk[0:C, 0:C] = wz2.T
    w2blk[C : 2 * C, C : C + CH] = 0.5 * wf2.T[:, 0:CH]
    w2blk[C : 2 * C, C + CH : 2 * C] = 0.5 * wf2.T[:, CH:C]
    w2blk = w2blk.astype(ml_dtypes.bfloat16)
    b1cat = np.concatenate([bz1, bf1]).astype(f32).reshape(-1, 1)
    # psZ rows = [zP_fwd(ch 0:32), zQ_fwd(ch 0:32), zP_bwd(ch 32:64), zQ_bwd]
    bz2p = np.concatenate([bz2, bz2]).astype(f32).reshape(-1, 1)
    bf2p = 0.5 * np.concatenate([bf2, bf2]).astype(f32).reshape(-1, 1)
    return dict(w1catT=w1catT, w2blk=w2blk, b1cat=b1cat, bz2p=bz2p, bf2p=bf2p)


def kernel(inputs, wz1, bz1, wz2, bz2, wf1, bf1, wf2, bf2):
    from concourse.bass_utils import run_bass_kernel_spmd

    if "nc" not in _built:
        _built["nc"] = _build()
    nc = _built["nc"]

    wd = _prep_weights(
        np.asarray(wz1), np.asarray(bz1), np.asarray(wz2), np.asarray(bz2),
        np.asarray(wf1), np.asarray(bf1), np.asarray(wf2), np.asarray(bf2),
    )
    xin = np.asarray(inputs, dtype=np.float32)
    in_maps = []
    for core in range(NCORES):
        shard = np.ascontiguousarray(xin[:, :, :, core * HL : (core + 1) * HL, :])
        m = {"x": shard}
        m.update(wd)
        in_maps.append(m)

    res = run_bass_kernel_spmd(nc, in_maps, core_ids=list(range(NCORES)))
    out = np.concatenate([r["y"] for r in res.results], axis=3)
    return out
